# revision 19
# baseline (speedup 1.0000x reference)
"""Flowformer attention Trainium2 kernel.

Full inputs -> full output. Internally: 8-way SPMD over (batch, head-pair):
core c handles batch c//4, heads {2*(c%4), 2*(c%4)+1}.

B=2, L=2048, D_MODEL=512, H=8, D=64. Everything fp32.

Algorithm (validated reformulation of the reference):
  q = sig(X_q Wq + bq), k = sig(X_k Wk + bk), v = X_v Wv + bv  (per head [L,64])
  chunked causal linear attention with C=128 chunks; all cumsum/dot-product
  chains expressed as masked-matrix matmuls:
    ATs = kc qc^T, As = qc kc^T ; masked with upper-tri (incl diag) U
    dotsi = ATm^T 1 + qc run_k          (= q_l . cumsum(k)_l)
    dotso = Am^T 1 + kc run_q
    dotcsrc = Am^T si_n + kc run_qs ; src = exp(dotcsrc/normal)
    dotcs | x = ATm^T [so_n | vv] + qc [run_ks | KV]
    x *= (1/dotsi) * sigmoid(dotcs/normal); out = x @ Wo (partial, host-summed)
  exp computed as sig(y)/sig(-y) so only the sigmoid ACT table set is used.
"""

import numpy as np

import concourse.bacc as bacc
import concourse.bass as bass
import concourse.mybir as mybir
from concourse import tile
from concourse.bass_utils import run_bass_kernel_spmd

F32 = mybir.dt.float32
AF = mybir.ActivationFunctionType
OP = mybir.AluOpType

B, L, DM, H = 2, 2048, 512, 8
D = DM // H          # 64 head dim
C = 128              # chunk
NCH = L // C         # 16 chunks
KB = DM // 128       # 4 k-blocks
HPC = 2              # heads per core
NCORES = 8


def build_nc():
    nc = bacc.Bacc("TRN2", target_bir_lowering=False, num_devices=NCORES)

    # ---- DRAM I/O ----
    xt_q = nc.dram_tensor("xt_q", [DM, L], F32, kind="ExternalInput")
    xt_k = nc.dram_tensor("xt_k", [DM, L], F32, kind="ExternalInput")
    xt_v = nc.dram_tensor("xt_v", [DM, L], F32, kind="ExternalInput")
    wq_d = nc.dram_tensor("wq", [DM, 128], F32, kind="ExternalInput")
    wk_d = nc.dram_tensor("wk", [DM, 128], F32, kind="ExternalInput")
    wv_d = nc.dram_tensor("wv", [DM, 128], F32, kind="ExternalInput")
    wo_d = nc.dram_tensor("wo", [128, DM], F32, kind="ExternalInput")
    bq_c_d = nc.dram_tensor("bq_col", [128, 1], F32, kind="ExternalInput")
    bk_c_d = nc.dram_tensor("bk_col", [128, 1], F32, kind="ExternalInput")
    bq_r_d = nc.dram_tensor("bq_row", [1, 128], F32, kind="ExternalInput")
    bk_r_d = nc.dram_tensor("bk_row", [1, 128], F32, kind="ExternalInput")
    bv_r_d = nc.dram_tensor("bv_row", [1, 128], F32, kind="ExternalInput")
    um_d = nc.dram_tensor("umask4", [128, 512], F32, kind="ExternalInput")
    id_d = nc.dram_tensor("identity", [128, 128], F32, kind="ExternalInput")
    oc_d = nc.dram_tensor("ones_col", [128, 1], F32, kind="ExternalInput")
    or_d = nc.dram_tensor("ones_row", [1, 128], F32, kind="ExternalInput")
    nn_d = nc.dram_tensor("normal_n", [128, NCH], F32, kind="ExternalInput")
    nr_d = nc.dram_tensor("normal_r", [128, NCH], F32, kind="ExternalInput")
    out_d = nc.dram_tensor("out", [L, DM], F32, kind="ExternalOutput")

    with tile.TileContext(nc) as tc:
        with (
            tc.tile_pool(name="const", bufs=1) as cp,
            tc.tile_pool(name="xt", bufs=1) as xp,
            tc.tile_pool(name="proj", bufs=1) as pp,
            tc.tile_pool(name="work", bufs=3) as wk,
            tc.tile_pool(name="state", bufs=2) as stp,
            tc.tile_pool(name="outp", bufs=3) as op_,
        ):
            # ---- consts / weights to SBUF ----
            umask = cp.tile([128, 512], F32)
            ident = cp.tile([128, 128], F32)
            ones_col = cp.tile([128, 1], F32)
            ones_row = cp.tile([1, 128], F32)
            normal_n = cp.tile([128, NCH], F32)
            normal_r = cp.tile([128, NCH], F32)
            wq = cp.tile([128, KB, 128], F32)
            wkt = cp.tile([128, KB, 128], F32)
            wv = cp.tile([128, KB, 128], F32)
            wo = cp.tile([128, DM], F32)
            bq_col = cp.tile([128, 1], F32)
            bk_col = cp.tile([128, 1], F32)
            bq_row = cp.tile([1, 128], F32)
            bk_row = cp.tile([1, 128], F32)
            bv_row = cp.tile([1, 128], F32)

            nc.sync.dma_start(umask[:], um_d[:, :])
            nc.sync.dma_start(ident[:], id_d[:, :])
            nc.sync.dma_start(ones_col[:], oc_d[:, :])
            nc.sync.dma_start(ones_row[:], or_d[:, :])
            nc.sync.dma_start(normal_n[:], nn_d[:, :])
            nc.sync.dma_start(normal_r[:], nr_d[:, :])
            for kb in range(KB):
                nc.sync.dma_start(wq[:, kb, :], wq_d[kb * 128:(kb + 1) * 128, :])
                nc.sync.dma_start(wkt[:, kb, :], wk_d[kb * 128:(kb + 1) * 128, :])
                nc.sync.dma_start(wv[:, kb, :], wv_d[kb * 128:(kb + 1) * 128, :])
            nc.sync.dma_start(wo[:], wo_d[:, :])
            nc.sync.dma_start(bq_col[:], bq_c_d[:, :])
            nc.sync.dma_start(bk_col[:], bk_c_d[:, :])
            nc.sync.dma_start(bq_row[:], bq_r_d[:, :])
            nc.sync.dma_start(bk_row[:], bk_r_d[:, :])
            nc.sync.dma_start(bv_row[:], bv_r_d[:, :])

            # ---- transposed inputs to SBUF ----
            xtq = xp.tile([128, KB, L], F32)
            xtk = xp.tile([128, KB, L], F32)
            xtv = xp.tile([128, KB, L], F32)
            for kb in range(KB):
                nc.sync.dma_start(xtq[:, kb, :], xt_q[kb * 128:(kb + 1) * 128, :])
            for kb in range(KB):
                nc.sync.dma_start(xtk[:, kb, :], xt_k[kb * 128:(kb + 1) * 128, :])
            for kb in range(KB):
                nc.sync.dma_start(xtv[:, kb, :], xt_v[kb * 128:(kb + 1) * 128, :])

            # ---- persistent projection outputs ----
            # per-head [64, L] so base_partition matches state tiles (base 0)
            qTh = [pp.tile([64, L], F32, name=f"qT{h}", tag=f"qT{h}") for h in range(HPC)]
            kTh = [pp.tile([64, L], F32, name=f"kT{h}", tag=f"kT{h}") for h in range(HPC)]
            q_sb = pp.tile([128, NCH, 128], F32)   # [l-in-chunk, chunk, head-col]
            k_sb = pp.tile([128, NCH, 128], F32)
            v_sb = pp.tile([128, NCH, 128], F32)
            x_all = pp.tile([128, NCH, 128], F32)

            # ---- projections ----
            with (
                tc.tile_pool(name="pjbig", bufs=2, space=bass.MemorySpace.PSUM) as pjb,
                tc.tile_pool(name="pjsm", bufs=3, space=bass.MemorySpace.PSUM) as pjs,
            ):
                # qT / kT : [cols(128), 512] per L-superchunk, split per head
                for (xt, w, bcol, dst) in ((xtq, wq, bq_col, qTh), (xtk, wkt, bk_col, kTh)):
                    for sc in range(L // 512):
                        ps = pjb.tile([128, 512], F32, tag="pjb")
                        for kb in range(KB):
                            nc.tensor.matmul(
                                ps[:], w[:, kb, :], xt[:, kb, sc * 512:(sc + 1) * 512],
                                start=(kb == 0), stop=(kb == KB - 1),
                            )
                        for h in range(HPC):
                            nc.scalar.activation(
                                dst[h][:, sc * 512:(sc + 1) * 512],
                                ps[64 * h:64 * (h + 1), :], AF.Sigmoid,
                                bias=bcol[64 * h:64 * (h + 1), :],
                            )
                # q/k/v in [l, col] layout, chunk by chunk
                for i in range(NCH):
                    for (xt, w, brow, act, dst) in (
                        (xtq, wq, bq_row, True, q_sb),
                        (xtk, wkt, bk_row, True, k_sb),
                        (xtv, wv, bv_row, False, v_sb),
                    ):
                        ps = pjs.tile([128, 128], F32, tag="pjs")
                        for kb in range(KB):
                            nc.tensor.matmul(
                                ps[:], xt[:, kb, i * 128:(i + 1) * 128], w[:, kb, :],
                                start=(kb == 0), stop=False,
                            )
                        nc.tensor.matmul(ps[:], ones_row[:], brow[:],
                                         start=False, stop=True)
                        if act:
                            nc.scalar.activation(dst[:, i, :], ps[:], AF.Sigmoid)
                        else:
                            nc.vector.tensor_copy(dst[:, i, :], ps[:])

            # ---- attention chunks ----
            with (
                tc.tile_pool(name="p1", bufs=2, space=bass.MemorySpace.PSUM) as p1p,
                tc.tile_pool(name="p2", bufs=2, space=bass.MemorySpace.PSUM) as p2p,
                tc.tile_pool(name="pst", bufs=1, space=bass.MemorySpace.PSUM) as pstp,
                tc.tile_pool(name="pout", bufs=2, space=bass.MemorySpace.PSUM) as poutp,
            ):
                R_prev = None
                carry_prev = None
                # P2 layout per head h (base = 68*h):
                #   +0 dotsi, +1 dotso, +2 dotcs, +3..66 x, +67 dotcsrc
                # cols 136:138 cumsrc(2 heads), 138:266 xT
                for i in range(NCH):
                    hb = [68 * h for h in range(HPC)]
                    qTc = [qTh[h][:, i * 128:(i + 1) * 128] for h in range(HPC)]
                    kTc = [kTh[h][:, i * 128:(i + 1) * 128] for h in range(HPC)]
                    qc = [q_sb[:, i, 64 * h:64 * (h + 1)] for h in range(HPC)]
                    kc = [k_sb[:, i, 64 * h:64 * (h + 1)] for h in range(HPC)]
                    vc = [v_sb[:, i, 64 * h:64 * (h + 1)] for h in range(HPC)]
                    nn = normal_n[:, i:i + 1]
                    nr = normal_r[:, i:i + 1]

                    # 1. ATs | As  (per head) -> P1
                    P1 = p1p.tile([128, 4, 128], F32, tag="p1")
                    for h in range(HPC):
                        nc.tensor.matmul(P1[:, 2 * h, :], kTc[h], qTc[h],
                                         start=True, stop=True)
                        nc.tensor.matmul(P1[:, 2 * h + 1, :], qTc[h], kTc[h],
                                         start=True, stop=True)
                    # 2. mask (is also the PSUM->SBUF move)
                    AM = wk.tile([128, 4, 128], F32, tag="am")
                    nc.vector.tensor_tensor(
                        AM.rearrange("p a b -> p (a b)"),
                        P1.rearrange("p a b -> p (a b)"),
                        umask[:], OP.mult,
                    )
                    ATm = [AM[:, 2 * h, :] for h in range(HPC)]
                    Am = [AM[:, 2 * h + 1, :] for h in range(HPC)]

                    P2 = p2p.tile([128, 272], F32, tag="p2")
                    # 3. dotsi / dotso
                    for h in range(HPC):
                        nc.tensor.matmul(P2[:, hb[h]:hb[h] + 1], ATm[h], ones_col[:],
                                         start=True, stop=(i == 0))
                        if i > 0:
                            nc.tensor.matmul(P2[:, hb[h]:hb[h] + 1], qTc[h],
                                             R_prev[:, hb[h]:hb[h] + 1],
                                             start=False, stop=True)
                        nc.tensor.matmul(P2[:, hb[h] + 1:hb[h] + 2], Am[h], ones_col[:],
                                         start=True, stop=(i == 0))
                        if i > 0:
                            nc.tensor.matmul(P2[:, hb[h] + 1:hb[h] + 2], kTc[h],
                                             R_prev[:, hb[h] + 66:hb[h] + 67],
                                             start=False, stop=True)
                    # 4. si/so chains
                    si_raw = wk.tile([128, 2], F32, tag="t2a")
                    si_n = wk.tile([128, 2], F32, tag="t2b")
                    so_raw = wk.tile([128, 2], F32, tag="t2c")
                    T_k = wk.tile([128, 2, 65], F32, tag="tk")   # [so_n | vv]
                    nc.vector.reciprocal(si_raw[:], P2[:, 0:69:68])
                    nc.vector.tensor_scalar(si_n[:], si_raw[:], nn, None, op0=OP.mult)
                    nc.vector.reciprocal(so_raw[:], P2[:, 1:70:68])
                    nc.vector.tensor_scalar(T_k[:, :, 0], so_raw[:], nn, None, op0=OP.mult)
                    # 5. dotcsrc
                    for h in range(HPC):
                        nc.tensor.matmul(P2[:, hb[h] + 67:hb[h] + 68], Am[h],
                                         si_n[:, h:h + 1], start=True, stop=(i == 0))
                        if i > 0:
                            nc.tensor.matmul(P2[:, hb[h] + 67:hb[h] + 68], kTc[h],
                                             R_prev[:, hb[h] + 67:hb[h] + 68],
                                             start=False, stop=True)
                    # 6. src = exp(dotcsrc/normal) = sig(y)/sig(-y)
                    t0 = wk.tile([128, 2], F32, tag="t2d")
                    sp = wk.tile([128, 2], F32, tag="t2e")
                    sm = wk.tile([128, 2], F32, tag="t2f")
                    smr = wk.tile([128, 2], F32, tag="t2g")
                    src = wk.tile([128, 2], F32, tag="t2h")
                    nc.vector.tensor_scalar(t0[:], P2[:, 67:136:68], nr, None, op0=OP.mult)
                    nc.scalar.activation(sp[:], t0[:], AF.Sigmoid)
                    nc.scalar.activation(sm[:], t0[:], AF.Sigmoid, scale=-1.0)
                    nc.vector.reciprocal(smr[:], sm[:])
                    nc.vector.tensor_tensor(src[:], sp[:], smr[:], OP.mult)
                    # 7. cumsrc (within chunk + carry)
                    nc.tensor.matmul(P2[:, 136:138], umask[:, 0:128], src[:],
                                     start=True, stop=(i == 0))
                    if i > 0:
                        nc.tensor.matmul(P2[:, 136:138], ones_row[:], carry_prev[:],
                                         start=False, stop=True)
                    rc = wk.tile([128, 2], F32, tag="t2i")
                    sc_raw = wk.tile([128, 2], F32, tag="t2j")
                    nc.vector.reciprocal(rc[:], P2[:, 136:138])
                    nc.vector.tensor_tensor(sc_raw[:], src[:], rc[:], OP.mult)
                    # 8. vv = v * sc_raw * normal
                    for h in range(HPC):
                        nc.vector.tensor_scalar(
                            T_k[:, h, 1:65], vc[h], sc_raw[:, h:h + 1], nn,
                            op0=OP.mult, op1=OP.mult,
                        )
                    # 9. dotcs | x
                    for h in range(HPC):
                        nc.tensor.matmul(P2[:, hb[h] + 2:hb[h] + 67], ATm[h],
                                         T_k[:, h, :], start=True, stop=(i == 0))
                        if i > 0:
                            nc.tensor.matmul(P2[:, hb[h] + 2:hb[h] + 67], qTc[h],
                                             R_prev[:, hb[h] + 1:hb[h] + 66],
                                             start=False, stop=True)
                    # 10. sink_alloc, scale, x out
                    sa = wk.tile([128, 2], F32, tag="t2l")
                    scale = wk.tile([128, 2], F32, tag="t2m")
                    nc.scalar.activation(sa[:], P2[:, 2:71:68], AF.Sigmoid, scale=nr)
                    nc.vector.tensor_tensor(scale[:], si_raw[:], sa[:], OP.mult)
                    for h in range(HPC):
                        nc.vector.tensor_scalar(
                            x_all[:, i, 64 * h:64 * (h + 1)],
                            P2[:, hb[h] + 3:hb[h] + 67],
                            scale[:, h:h + 1], None, op0=OP.mult,
                        )
                    # carry for next chunk: running column-sum of src
                    # (partition-127 PSUM reads are illegal, so colsum by matmul)
                    carry = wk.tile([1, 2], F32, tag="t2k")
                    # 11. state updates -> scratch psum; R = R_prev + scratch
                    SCR = pstp.tile([64, 138], F32, tag="scr")
                    nc.tensor.matmul(SCR[0:1, 136:138], ones_col[:], src[:],
                                     start=True, stop=True)
                    if i == 0:
                        nc.vector.tensor_copy(carry[:], SCR[0:1, 136:138])
                    else:
                        nc.vector.tensor_tensor(carry[:], SCR[0:1, 136:138],
                                                carry_prev[:], OP.add)
                    for h in range(HPC):
                        nc.tensor.matmul(SCR[:, hb[h]:hb[h] + 1], kc[h], ones_col[:],
                                         start=True, stop=True)
                        nc.tensor.matmul(SCR[:, hb[h] + 1:hb[h] + 66], kc[h],
                                         T_k[:, h, :], start=True, stop=True)
                        nc.tensor.matmul(SCR[:, hb[h] + 66:hb[h] + 67], qc[h],
                                         ones_col[:], start=True, stop=True)
                        nc.tensor.matmul(SCR[:, hb[h] + 67:hb[h] + 68], qc[h],
                                         si_n[:, h:h + 1], start=True, stop=True)
                    R = stp.tile([64, 136], F32, tag="R")
                    if i == 0:
                        nc.vector.tensor_copy(R[:], SCR[:, 0:136])
                    else:
                        nc.vector.tensor_tensor(R[:], SCR[:, 0:136], R_prev[:], OP.add)
                    R_prev = R
                    carry_prev = carry
                    # 12. output projection for this chunk
                    nc.tensor.transpose(P2[:, 138:266], x_all[:, i, :], ident[:])
                    xTs = op_.tile([128, 128], F32, tag="xts")
                    nc.scalar.copy(xTs[:], P2[:, 138:266])
                    PO = poutp.tile([128, 512], F32, tag="po")
                    nc.tensor.matmul(PO[:], xTs[:], wo[:], start=True, stop=True)
                    osb = op_.tile([128, 512], F32, tag="osb")
                    nc.scalar.copy(osb[:], PO[:])
                    nc.sync.dma_start(out_d[i * 128:(i + 1) * 128, :], osb[:])

    nc.compile()
    return nc


_NC_CACHE = None


def _get_nc():
    global _NC_CACHE
    if _NC_CACHE is None:
        _NC_CACHE = build_nc()
    return _NC_CACHE


def make_in_maps(queries, keys, values, Wq, bq, Wk, bk, Wv, bv, Wo, bo):
    f = np.float32
    U = np.triu(np.ones((128, 128), f))
    consts = {
        "umask4": np.ascontiguousarray(np.tile(U, (1, 4))),
        "identity": np.eye(128, dtype=f),
        "ones_col": np.ones((128, 1), f),
        "ones_row": np.ones((1, 128), f),
    }
    nn = np.zeros((128, NCH), f)
    for i in range(NCH):
        nn[:, i] = np.arange(i * 128, (i + 1) * 128, dtype=f) + 1.0
    consts["normal_n"] = nn
    consts["normal_r"] = (1.0 / nn).astype(f)

    xts = {}
    for b in range(B):
        xts[b] = {
            "xt_q": np.ascontiguousarray(np.asarray(queries[b], f).T),
            "xt_k": np.ascontiguousarray(np.asarray(keys[b], f).T),
            "xt_v": np.ascontiguousarray(np.asarray(values[b], f).T),
        }
    in_maps = []
    for c in range(NCORES):
        b = c // 4
        h0 = 2 * (c % 4)
        cs = slice(64 * h0, 64 * h0 + 128)
        m = dict(consts)
        m.update(xts[b])
        m["wq"] = np.ascontiguousarray(np.asarray(Wq, f)[:, cs])
        m["wk"] = np.ascontiguousarray(np.asarray(Wk, f)[:, cs])
        m["wv"] = np.ascontiguousarray(np.asarray(Wv, f)[:, cs])
        m["wo"] = np.ascontiguousarray(np.asarray(Wo, f)[cs, :])
        m["bq_col"] = np.asarray(bq, f)[cs].reshape(128, 1).copy()
        m["bk_col"] = np.asarray(bk, f)[cs].reshape(128, 1).copy()
        m["bq_row"] = np.asarray(bq, f)[cs].reshape(1, 128).copy()
        m["bk_row"] = np.asarray(bk, f)[cs].reshape(1, 128).copy()
        m["bv_row"] = np.asarray(bv, f)[cs].reshape(1, 128).copy()
        in_maps.append(m)
    return in_maps


def kernel(queries, keys, values, Wq, bq, Wk, bk, Wv, bv, Wo, bo, _trace=False):
    nc = _get_nc()
    in_maps = make_in_maps(queries, keys, values, Wq, bq, Wk, bk, Wv, bv, Wo, bo)
    res = run_bass_kernel_spmd(nc, in_maps, core_ids=list(range(NCORES)),
                               trace=_trace)
    out = np.zeros((B, L, DM), np.float32)
    for b in range(B):
        acc = np.zeros((L, DM), np.float32)
        for c in range(4 * b, 4 * b + 4):
            acc += res.results[c]["out"]
        out[b] = acc + np.asarray(bo, np.float32)
    if _trace:
        return out, res
    return out


# revision 24
# speedup vs baseline: 2.0766x; 2.0766x over previous
"""Flowformer attention Trainium2 kernel.

Full inputs -> full output. Internally: 8-way SPMD over (batch, head-pair):
core c handles batch c//4, heads {2*(c%4), 2*(c%4)+1}.

B=2, L=2048, D_MODEL=512, H=8, D=64.

Matmul operands are bf16 (fp32 matmuls lower to LO/HI instruction pairs at
~1/4 throughput); all accumulation stays fp32 in PSUM, the scalar
flow-conservation chain stays fp32, and the output is fp32.

Algorithm (validated reformulation of the reference):
  q = sig(X_q Wq + bq), k = sig(X_k Wk + bk), v = X_v Wv + bv  (per head [L,64])
  chunked causal linear attention with C=128 chunks; all cumsum/dot-product
  chains are masked-matrix matmuls:
    ATs = kc qc^T, As = qc kc^T ; masked with upper-tri (incl diag) U
    dotsi = ATm^T 1 + qc run_k          (= q_l . cumsum(k)_l)
    dotso = Am^T 1 + kc run_q
    dotcsrc = Am^T si_n + kc run_qs ; src = exp(dotcsrc/normal)
    dotcs | x = ATm^T [so_n | vv] + qc [run_ks | KV]
    x *= (1/dotsi) * sigmoid(dotcs/normal); out = x @ Wo (partial, host-summed)
  exp is computed as sig(y)/sig(-y) so only the sigmoid ACT table set is
  ever loaded. Running state accumulates in a persistent PSUM bank.
"""

import numpy as np
import ml_dtypes

import concourse.bacc as bacc
import concourse.bass as bass
import concourse.mybir as mybir
from concourse import tile
from concourse.bass_utils import run_bass_kernel_spmd
from concourse.tile_rust import add_dep_helper

F32 = mybir.dt.float32
BF = mybir.dt.bfloat16
AF = mybir.ActivationFunctionType
OP = mybir.AluOpType

B, L, DM, H = 2, 2048, 512, 8
D = DM // H          # 64 head dim
C = 128              # chunk
NCH = L // C         # 16 chunks
KB = DM // 128       # 4 k-blocks
HPC = 2              # heads per core
NCORES = 8


def build_nc():
    nc = bacc.Bacc("TRN2", target_bir_lowering=False, num_devices=NCORES)

    # ---- DRAM I/O ----
    xt_q = nc.dram_tensor("xt_q", [DM, L], BF, kind="ExternalInput")
    xt_k = nc.dram_tensor("xt_k", [DM, L], BF, kind="ExternalInput")
    xt_v = nc.dram_tensor("xt_v", [DM, L], BF, kind="ExternalInput")
    wq_d = nc.dram_tensor("wq", [DM, 128], BF, kind="ExternalInput")
    wk_d = nc.dram_tensor("wk", [DM, 128], BF, kind="ExternalInput")
    wv_d = nc.dram_tensor("wv", [DM, 128], BF, kind="ExternalInput")
    wo_d = nc.dram_tensor("wo", [128, DM], BF, kind="ExternalInput")
    bq_c_d = nc.dram_tensor("bq_col", [128, 1], F32, kind="ExternalInput")
    bk_c_d = nc.dram_tensor("bk_col", [128, 1], F32, kind="ExternalInput")
    bq_r_d = nc.dram_tensor("bq_row", [1, 128], BF, kind="ExternalInput")
    bk_r_d = nc.dram_tensor("bk_row", [1, 128], BF, kind="ExternalInput")
    bv_r_d = nc.dram_tensor("bv_row", [1, 128], BF, kind="ExternalInput")
    um_d = nc.dram_tensor("umask4", [128, 512], F32, kind="ExternalInput")
    ub_d = nc.dram_tensor("uones_bf", [128, 128], BF, kind="ExternalInput")
    id_d = nc.dram_tensor("identity", [128, 128], F32, kind="ExternalInput")
    oc_d = nc.dram_tensor("ones_col", [128, 1], BF, kind="ExternalInput")
    or_d = nc.dram_tensor("ones_row", [1, 128], BF, kind="ExternalInput")
    nn_d = nc.dram_tensor("normal_n", [128, NCH], F32, kind="ExternalInput")
    nr_d = nc.dram_tensor("normal_r", [128, NCH], F32, kind="ExternalInput")
    out_d = nc.dram_tensor("out", [L, DM], F32, kind="ExternalOutput")

    with tile.TileContext(nc) as tc:
        with (
            tc.tile_pool(name="const", bufs=1) as cp,
            tc.tile_pool(name="xt", bufs=1) as xp,
            tc.tile_pool(name="proj", bufs=1) as pp,
            tc.tile_pool(name="work", bufs=3) as wk,
            tc.tile_pool(name="state", bufs=2) as stp,
            tc.tile_pool(name="outp", bufs=3) as op_,
        ):
            # ---- consts / weights to SBUF ----
            umask = cp.tile([128, 512], F32)
            uones = cp.tile([128, 128], BF)
            ident = cp.tile([128, 128], F32)
            ones_col = cp.tile([128, 1], BF)
            ones_row = cp.tile([1, 128], BF)
            normal_n = cp.tile([128, NCH], F32)
            normal_r = cp.tile([128, NCH], F32)
            wq = cp.tile([128, KB, 128], BF)
            wkt = cp.tile([128, KB, 128], BF)
            wv = cp.tile([128, KB, 128], BF)
            wo = cp.tile([128, DM], BF)
            bq_col = cp.tile([128, 1], F32)
            bk_col = cp.tile([128, 1], F32)
            bq_row = cp.tile([1, 128], BF)
            bk_row = cp.tile([1, 128], BF)
            bv_row = cp.tile([1, 128], BF)

            nc.sync.dma_start(umask[:], um_d[:, :])
            nc.sync.dma_start(uones[:], ub_d[:, :])
            nc.sync.dma_start(ident[:], id_d[:, :])
            nc.sync.dma_start(ones_col[:], oc_d[:, :])
            nc.sync.dma_start(ones_row[:], or_d[:, :])
            nc.sync.dma_start(normal_n[:], nn_d[:, :])
            nc.sync.dma_start(normal_r[:], nr_d[:, :])
            for kb in range(KB):
                nc.sync.dma_start(wq[:, kb, :], wq_d[kb * 128:(kb + 1) * 128, :])
                nc.sync.dma_start(wkt[:, kb, :], wk_d[kb * 128:(kb + 1) * 128, :])
                nc.sync.dma_start(wv[:, kb, :], wv_d[kb * 128:(kb + 1) * 128, :])
            nc.sync.dma_start(wo[:], wo_d[:, :])
            nc.sync.dma_start(bq_col[:], bq_c_d[:, :])
            nc.sync.dma_start(bk_col[:], bk_c_d[:, :])
            nc.sync.dma_start(bq_row[:], bq_r_d[:, :])
            nc.sync.dma_start(bk_row[:], bk_r_d[:, :])
            nc.sync.dma_start(bv_row[:], bv_r_d[:, :])

            # ---- transposed inputs to SBUF ----
            xtq = xp.tile([128, KB, L], BF)
            xtk = xp.tile([128, KB, L], BF)
            xtv = xp.tile([128, KB, L], BF)
            for kb in range(KB):
                nc.sync.dma_start(xtq[:, kb, :], xt_q[kb * 128:(kb + 1) * 128, :])
            for kb in range(KB):
                nc.sync.dma_start(xtk[:, kb, :], xt_k[kb * 128:(kb + 1) * 128, :])
            for kb in range(KB):
                nc.sync.dma_start(xtv[:, kb, :], xt_v[kb * 128:(kb + 1) * 128, :])

            # ---- persistent projection outputs (bf16) ----
            # per-head [64, L] so base_partition matches state tiles (base 0)
            qTh = [pp.tile([64, L], BF, name=f"qT{h}", tag=f"qT{h}") for h in range(HPC)]
            kTh = [pp.tile([64, L], BF, name=f"kT{h}", tag=f"kT{h}") for h in range(HPC)]
            q_sb = pp.tile([128, NCH, 128], BF)   # [l-in-chunk, chunk, head-col]
            k_sb = pp.tile([128, NCH, 128], BF)
            v_sb = pp.tile([128, NCH, 128], BF)
            x_all = pp.tile([128, NCH, 128], F32)

            # ---- projections ----
            with (
                tc.tile_pool(name="pjbig", bufs=2, space=bass.MemorySpace.PSUM) as pjb,
                tc.tile_pool(name="pjsm", bufs=3, space=bass.MemorySpace.PSUM) as pjs,
            ):
                # qT / kT : [cols(128), 512] per L-superchunk, split per head
                for (xt, w, bcol, dst) in ((xtq, wq, bq_col, qTh), (xtk, wkt, bk_col, kTh)):
                    for sc in range(L // 512):
                        ps = pjb.tile([128, 512], F32, tag="pjb")
                        for kb in range(KB):
                            nc.tensor.matmul(
                                ps[:], w[:, kb, :], xt[:, kb, sc * 512:(sc + 1) * 512],
                                start=(kb == 0), stop=(kb == KB - 1),
                            )
                        for h in range(HPC):
                            nc.scalar.activation(
                                dst[h][:, sc * 512:(sc + 1) * 512],
                                ps[64 * h:64 * (h + 1), :], AF.Sigmoid,
                                bias=bcol[64 * h:64 * (h + 1), :],
                            )
                # q/k/v in [l, col] layout, chunk by chunk
                for i in range(NCH):
                    for (xt, w, brow, act, dst) in (
                        (xtq, wq, bq_row, True, q_sb),
                        (xtk, wkt, bk_row, True, k_sb),
                        (xtv, wv, bv_row, False, v_sb),
                    ):
                        ps = pjs.tile([128, 128], F32, tag="pjs")
                        for kb in range(KB):
                            nc.tensor.matmul(
                                ps[:], xt[:, kb, i * 128:(i + 1) * 128], w[:, kb, :],
                                start=(kb == 0), stop=False,
                            )
                        nc.tensor.matmul(ps[:], ones_row[:], brow[:],
                                         start=False, stop=True)
                        if act:
                            nc.scalar.activation(dst[:, i, :], ps[:], AF.Sigmoid)
                        else:
                            nc.vector.tensor_copy(dst[:, i, :], ps[:])

            # ---- attention chunks ----
            with (
                tc.tile_pool(name="p1", bufs=2, space=bass.MemorySpace.PSUM) as p1p,
                tc.tile_pool(name="p2", bufs=2, space=bass.MemorySpace.PSUM) as p2p,
                tc.tile_pool(name="pst", bufs=1, space=bass.MemorySpace.PSUM) as pstp,
                tc.tile_pool(name="pout", bufs=2, space=bass.MemorySpace.PSUM) as poutp,
            ):
                # persistent running-state accumulator:
                # per head h: [:, h, 0]=run_k, 1=run_ks, 2:66=KV, 66=run_q,
                # 67=run_qs ; [0:1, 0, 136:138]-> src carry colsum
                SCR = pstp.tile([64, 2, 138], F32)
                scr_start = [None]

                def scrmm(out, lhsT, rhs):
                    bi = nc.tensor.matmul(out, lhsT, rhs,
                                          start=(scr_start[0] is None),
                                          stop=False, skip_group_check=True)
                    if scr_start[0] is None:
                        scr_start[0] = bi.ins
                    else:
                        add_dep_helper(bi.ins, scr_start[0],
                                       reason="SCR bank has_written clear first")
                R_prev = None
                carry_prev = None
                # P2 layout per head h (base = 68*h):
                #   +0 dotsi, +1..65 dotcs|x, +66 dotso, +67 dotcsrc
                # cols 136:138 cumsrc(2 heads), 138:266 xT
                for i in range(NCH):
                    hb = [68 * h for h in range(HPC)]
                    qTc = [qTh[h][:, i * 128:(i + 1) * 128] for h in range(HPC)]
                    kTc = [kTh[h][:, i * 128:(i + 1) * 128] for h in range(HPC)]
                    qc = [q_sb[:, i, 64 * h:64 * (h + 1)] for h in range(HPC)]
                    kc = [k_sb[:, i, 64 * h:64 * (h + 1)] for h in range(HPC)]
                    vc = [v_sb[:, i, 64 * h:64 * (h + 1)] for h in range(HPC)]
                    nn = normal_n[:, i:i + 1]
                    nr = normal_r[:, i:i + 1]

                    P2 = p2p.tile([128, 272], F32, tag="p2")
                    # PSUM group discipline: exactly ONE start=True per P2
                    # buffer per chunk (clears the bank's has_written bits);
                    # every other matmul start=False = overwrite-on-first-
                    # touch / accumulate-on-second. skip_group_check bypasses
                    # the sim's strict bracket checker (per-byte pending-zero
                    # execution stays exact).
                    p2start = [None]

                    def p2mm(out, lhsT, rhs, is_transpose=False):
                        bi = nc.tensor.matmul(out, lhsT, rhs,
                                              start=(p2start[0] is None),
                                              stop=False,
                                              is_transpose=is_transpose,
                                              skip_group_check=True)
                        if p2start[0] is None:
                            p2start[0] = bi.ins
                        else:
                            add_dep_helper(bi.ins, p2start[0],
                                           reason="P2 bank has_written clear first")

                    # 0. state-dependent partial sums first (only need R_{i-1})
                    #    [dotsi | dotcs | inter] and [dotso | dotcsrc]
                    if i > 0:
                        for h in range(HPC):
                            p2mm(P2[:, hb[h]:hb[h] + 66], qTc[h],
                                 R_prev[h][:, 0:66])
                            p2mm(P2[:, hb[h] + 66:hb[h] + 68], kTc[h],
                                 R_prev[h][:, 66:68])
                    # 1. ATs | As  (per head) -> P1
                    P1 = p1p.tile([128, 4, 128], F32, tag="p1")
                    for h in range(HPC):
                        nc.tensor.matmul(P1[:, 2 * h, :], kTc[h], qTc[h],
                                         start=True, stop=True)
                        nc.tensor.matmul(P1[:, 2 * h + 1, :], qTc[h], kTc[h],
                                         start=True, stop=True)
                    # 2. mask (is also the PSUM->SBUF move), cast to bf16
                    AM = wk.tile([128, 4, 128], BF, tag="am")
                    nc.vector.tensor_tensor(
                        AM.rearrange("p a b -> p (a b)"),
                        P1.rearrange("p a b -> p (a b)"),
                        umask[:], OP.mult,
                    )
                    ATm = [AM[:, 2 * h, :] for h in range(HPC)]
                    Am = [AM[:, 2 * h + 1, :] for h in range(HPC)]

                    # 3. dotsi / dotso intra parts
                    for h in range(HPC):
                        p2mm(P2[:, hb[h]:hb[h] + 1], ATm[h], ones_col[:])
                        p2mm(P2[:, hb[h] + 66:hb[h] + 67], Am[h], ones_col[:])
                    # 4. si/so chains
                    si_raw = wk.tile([128, 2], F32, tag="t2a")
                    si_n = wk.tile([128, 2], BF, tag="t2b")
                    so_raw = wk.tile([128, 2], F32, tag="t2c")
                    T_k = wk.tile([128, 2, 65], BF, tag="tk")   # [so_n | vv]
                    nc.vector.reciprocal(si_raw[:], P2[:, 0:69:68])
                    nc.vector.tensor_scalar(si_n[:], si_raw[:], nn, None, op0=OP.mult)
                    nc.vector.reciprocal(so_raw[:], P2[:, 66:135:68])
                    nc.vector.tensor_scalar(T_k[:, :, 0], so_raw[:], nn, None, op0=OP.mult)
                    # 5. dotcsrc intra part
                    for h in range(HPC):
                        p2mm(P2[:, hb[h] + 67:hb[h] + 68], Am[h], si_n[:, h:h + 1])
                    # 6. src = exp(dotcsrc/normal) = sig(y)/sig(-y)
                    t0 = wk.tile([128, 2], F32, tag="t2d")
                    sp = wk.tile([128, 2], F32, tag="t2e")
                    sm = wk.tile([128, 2], F32, tag="t2f")
                    smr = wk.tile([128, 2], F32, tag="t2g")
                    src = wk.tile([128, 2], F32, tag="t2h")
                    src_bf = wk.tile([128, 2], BF, tag="t2hb")
                    nc.vector.tensor_scalar(t0[:], P2[:, 67:136:68], nr, None, op0=OP.mult)
                    nc.scalar.activation(sp[:], t0[:], AF.Sigmoid)
                    nc.scalar.activation(sm[:], t0[:], AF.Sigmoid, scale=-1.0)
                    nc.vector.reciprocal(smr[:], sm[:])
                    nc.vector.tensor_tensor(src[:], sp[:], smr[:], OP.mult)
                    nc.vector.tensor_copy(src_bf[:], src[:])
                    # 7. cumsrc (within chunk + carry)
                    p2mm(P2[:, 136:138], uones[:], src_bf[:])
                    if i > 0:
                        p2mm(P2[:, 136:138], ones_row[:], carry_prev[:])
                    rc = wk.tile([128, 2], F32, tag="t2i")
                    sc_n = wk.tile([128, 2], F32, tag="t2j")
                    nc.vector.reciprocal(rc[:], P2[:, 136:138])
                    nc.vector.tensor_tensor(sc_n[:], src[:], rc[:], OP.mult)
                    nc.vector.tensor_scalar(sc_n[:], sc_n[:], nn, None, op0=OP.mult)
                    # 8. vv = v * sc_n   (ACT copy with per-partition scale)
                    for h in range(HPC):
                        nc.scalar.activation(T_k[:, h, 1:65], vc[h], AF.Copy,
                                             scale=sc_n[:, h:h + 1])
                    # 9. dotcs | intra-x
                    for h in range(HPC):
                        p2mm(P2[:, hb[h] + 1:hb[h] + 66], ATm[h], T_k[:, h, :])
                    # 10. sink_alloc, scale, x out
                    sa = wk.tile([128, 2], F32, tag="t2l")
                    scale = wk.tile([128, 2], F32, tag="t2m")
                    nc.scalar.activation(sa[:], P2[:, 1:70:68], AF.Sigmoid, scale=nr)
                    nc.vector.tensor_tensor(scale[:], si_raw[:], sa[:], OP.mult)
                    for h in range(HPC):
                        nc.scalar.activation(x_all[:, i, 64 * h:64 * (h + 1)],
                                             P2[:, hb[h] + 2:hb[h] + 66], AF.Copy,
                                             scale=scale[:, h:h + 1])
                    # 11. state updates accumulate in persistent PSUM
                    for h in range(HPC):
                        scrmm(SCR[:, h, 0:1], kc[h], ones_col[:])
                        scrmm(SCR[:, h, 1:66], kc[h], T_k[:, h, :])
                        scrmm(SCR[:, h, 66:67], qc[h], ones_col[:])
                        scrmm(SCR[:, h, 67:68], qc[h], si_n[:, h:h + 1])
                    # carry for next chunk: running column-sum of src (PSUM acc)
                    # (emitted after the 64-partition state matmuls so the
                    # bank's start=True matmul spans all state partitions)
                    carry = wk.tile([1, 2], BF, tag="t2k")
                    scrmm(SCR[0:1, 0, 136:138], ones_col[:], src_bf[:])
                    nc.vector.tensor_copy(carry[:], SCR[0:1, 0, 136:138])
                    R = [stp.tile([64, 68], BF, name=f"R{h}", tag=f"R{h}")
                         for h in range(HPC)]
                    for h in range(HPC):
                        nc.vector.tensor_copy(R[h][:], SCR[:, h, 0:68])
                    R_prev = R
                    carry_prev = carry
                    # 12. output projection for this chunk
                    p2mm(P2[:, 138:266], x_all[:, i, :], ident[:],
                         is_transpose=True)
                    xTs = op_.tile([128, 128], BF, tag="xts")
                    nc.scalar.copy(xTs[:], P2[:, 138:266])
                    PO = poutp.tile([128, 512], F32, tag="po")
                    nc.tensor.matmul(PO[:], xTs[:], wo[:], start=True, stop=True)
                    osb = op_.tile([128, 512], F32, tag="osb")
                    nc.scalar.copy(osb[:], PO[:])
                    nc.sync.dma_start(out_d[i * 128:(i + 1) * 128, :], osb[:])

    nc.compile()
    return nc


_NC_CACHE = None


def _get_nc():
    global _NC_CACHE
    if _NC_CACHE is None:
        _NC_CACHE = build_nc()
    return _NC_CACHE


def make_in_maps(queries, keys, values, Wq, bq, Wk, bk, Wv, bv, Wo, bo):
    f = np.float32
    bf = ml_dtypes.bfloat16
    U = np.triu(np.ones((128, 128), f))
    consts = {
        "umask4": np.ascontiguousarray(np.tile(U, (1, 4))),
        "uones_bf": U.astype(bf),
        "identity": np.eye(128, dtype=f),
        "ones_col": np.ones((128, 1), bf),
        "ones_row": np.ones((1, 128), bf),
    }
    nn = np.zeros((128, NCH), f)
    for i in range(NCH):
        nn[:, i] = np.arange(i * 128, (i + 1) * 128, dtype=f) + 1.0
    consts["normal_n"] = nn
    consts["normal_r"] = (1.0 / nn).astype(f)

    xts = {}
    for b in range(B):
        xts[b] = {
            "xt_q": np.ascontiguousarray(np.asarray(queries[b], f).T).astype(bf),
            "xt_k": np.ascontiguousarray(np.asarray(keys[b], f).T).astype(bf),
            "xt_v": np.ascontiguousarray(np.asarray(values[b], f).T).astype(bf),
        }
    in_maps = []
    for c in range(NCORES):
        b = c // 4
        h0 = 2 * (c % 4)
        cs = slice(64 * h0, 64 * h0 + 128)
        m = dict(consts)
        m.update(xts[b])
        m["wq"] = np.ascontiguousarray(np.asarray(Wq, f)[:, cs]).astype(bf)
        m["wk"] = np.ascontiguousarray(np.asarray(Wk, f)[:, cs]).astype(bf)
        m["wv"] = np.ascontiguousarray(np.asarray(Wv, f)[:, cs]).astype(bf)
        m["wo"] = np.ascontiguousarray(np.asarray(Wo, f)[cs, :]).astype(bf)
        m["bq_col"] = np.asarray(bq, f)[cs].reshape(128, 1).copy()
        m["bk_col"] = np.asarray(bk, f)[cs].reshape(128, 1).copy()
        m["bq_row"] = np.asarray(bq, f)[cs].reshape(1, 128).astype(bf)
        m["bk_row"] = np.asarray(bk, f)[cs].reshape(1, 128).astype(bf)
        m["bv_row"] = np.asarray(bv, f)[cs].reshape(1, 128).astype(bf)
        in_maps.append(m)
    return in_maps


def kernel(queries, keys, values, Wq, bq, Wk, bk, Wv, bv, Wo, bo, _trace=False):
    nc = _get_nc()
    in_maps = make_in_maps(queries, keys, values, Wq, bq, Wk, bk, Wv, bv, Wo, bo)
    res = run_bass_kernel_spmd(nc, in_maps, core_ids=list(range(NCORES)),
                               trace=_trace)
    out = np.zeros((B, L, DM), np.float32)
    for b in range(B):
        acc = np.zeros((L, DM), np.float32)
        for c in range(4 * b, 4 * b + 4):
            acc += res.results[c]["out"]
        out[b] = acc + np.asarray(bo, np.float32)
    if _trace:
        return out, res
    return out
